# revision 13
# baseline (speedup 1.0000x reference)
"""DTNN layer kernel for Trainium2 (8 NeuronCores).

Math: out[b,i,o] = sum_j sum_h Wfc[o,h] * hx[b,i,h] * hd[b,i,j,h]
with hx = x@Wcf.T + bcf, hd = dist@Wdf.T + bdf.
Since Wfc/Wdf are linear, the j-sum commutes:
    ds[b,i,d]  = sum_j dist[b,i,j,d]                  (memory-bound reduction)
    out[b,i,:] = ((x@Wcf.T + bcf) * (ds@Wdf.T + N*bdf)) @ Wfc.T
So the kernel streams `distance` once (134MB) and does a few 128x128 matmuls.

Sharding: flatten (B,N) -> 1024 i-rows, 128 rows per core; no cross-core comms.

v4 design (v1 ~70-75us, v2 65.9us; NTFF traces):
- v2's floor was SDMA engine 15 (E79; hard-wired to SBUF partitions 92-95 &
  124-127 by the partition->AXI-port map): it sustains ~22GB/s vs ~26.8 for
  its peers, and with bytes split exactly 1/16 per engine its FIFO alone set
  the stream end (the last ~8us of the stream was E79 draining solo).
- v4 rebalances ~12% of E79's bytes away host-side (matching its measured
  speed deficit): full-width [128,.] tiles carry j in [32,256) only.
  j in [0,32) arrives as: rows 0-91 naturally ([92,4096] tile, base 0);
  rows 96-123 packed wide as [56,2048] at base partition 32 (PE operands
  must sit at base 0/32/64); slow rows 92-95/124-127 packed as [16,2048] at
  base 0. Packed tiles are folded per-partition and their row-sums injected
  into the right dsT columns with masked matmuls on the idle PE (stationary
  = fold result, moving = 0/1 mask, accumulating into dsT PSUM).
- Everything else as v2: bf16 mid-stage folds (2x DVE), bf16 1-pass PE
  matmuls (weights/x pre-cast host-side), per-chunk transpose-accumulate of
  ds^T in PSUM, mirrored output (out^T = WfcT^T @ sT) with a constant
  stationary, ACT engine for PSUM->SBUF copies, tapered 16/8/8-j last tiles.
Numpy-simulated rel err of the bf16 scheme: ~4e-3 (gate 2e-2).
"""

import numpy as np
from ml_dtypes import bfloat16

import concourse.bass as bass
import concourse.bacc as bacc
import concourse.mybir as mybir
from concourse.tile import TileContext
from concourse.bass_utils import run_bass_kernel_spmd

B, N, D, H = 4, 256, 128, 128
NCORES = 8
ROWS = B * N // NCORES  # 128 i-rows per core
FP = mybir.dt.float32
BF = mybir.dt.bfloat16

SLOW = [92, 93, 94, 95, 124, 125, 126, 127]  # partitions served by SDMA E15
JOFF = 32            # j in [0,JOFF) is delivered via the special early tiles
FULL_SIZES = [32, 32, 32, 32, 32, 32, 16, 8, 8]   # j in [32,256)
assert sum(FULL_SIZES) == N - JOFF
PKCOLS = 16 * D      # packed tiles: 16 j-slices (2048 cols) per partition

# dist DRAM layout per core, [128, 34816] fp32:
#   cols [0, 4096):        j in [0,32)   (rows 92-95/124-127: pad)
#   cols [4096, 32768):    j in [32,256) for all rows
#   cols [32768, 34816):   packed: rows 0-15  = slow rows' j in [0,32)
#                          (row SLOW[p//2], j-slice p%2);
#                          rows 64-119 = rows 96-123's j in [0,32)
#                          (row 96+(p-64)//2, j-slice (p-64)%2; base 64 —
#                          DVE ops starting at partition 32 may span <=32)
DCOLS = 34816

CB_XT = 0
CB_WCF = 128
CB_WDF = 256
CB_WFC = 384
CB_BCFR = 512   # partition 0: bcf row (1, H)
CB_ONES = 640   # partition 0: ones row (1, ROWS)
CB_TOT = 768

CF_EYE = 0
CF_BDFN = 128   # per-partition column (H, 1) = N * bdf
CF_M1 = 129     # [16,4] masks: M1[p,c] = (p//2 == c), M2[p,c] = (p//2 == 4+c)
CF_M2 = 133
CF_M56 = 137    # [56,28] mask at partitions 32-87: M[p,c] = ((p-32)//2 == c)
CF_TOT = 165


def build_nc():
    nc = bacc.Bacc("TRN2", target_bir_lowering=False)
    dist = nc.declare_dram_parameter("dist", [ROWS, DCOLS], FP, isOutput=False)
    cstb = nc.declare_dram_parameter("cstb", [128, CB_TOT], BF, isOutput=False)
    cstf = nc.declare_dram_parameter("cstf", [128, CF_TOT], FP, isOutput=False)
    # out^T stored as two stacked i-halves, each a contiguous [D, 64] block
    out = nc.declare_dram_parameter("out", [2 * D, ROWS // 2], FP,
                                    isOutput=True)

    with TileContext(nc) as tc:
        with (
            tc.tile_pool(name="const", bufs=1) as cpool,
            tc.tile_pool(name="dist", bufs=1) as dpool,
            tc.tile_pool(name="scratch", bufs=1) as spool,
            tc.tile_pool(name="work", bufs=1) as wpool,
            tc.tile_pool(name="psum", bufs=1, space="PSUM") as ppool,
        ):
            # ---- DMA schedule (sync HWDGE ring, in order) ----
            # wA first so DVE can start early; fw0 second so E79 (which only
            # carries full-width bytes) starts almost immediately.
            wa = dpool.tile([92, JOFF * D], FP, tag="wa")
            nc.sync.dma_start(out=wa[:], in_=dist[0:92, 0:JOFF * D])
            ftiles = []
            foff = JOFF
            def full_dma(k, jn, foff):
                t = dpool.tile([ROWS, jn * D], FP, tag=f"fw{k}", name=f"fw{k}")
                nc.sync.dma_start(out=t[:],
                                  in_=dist[:, foff * D:(foff + jn) * D])
                return t
            ftiles.append(full_dma(0, FULL_SIZES[0], foff))
            foff += FULL_SIZES[0]
            pct = dpool.tile([120, PKCOLS], FP, tag="pct")
            nc.sync.dma_start(out=pct[64:120, :], in_=dist[64:120, 32768:DCOLS])
            gt = dpool.tile([16, PKCOLS], FP, tag="gt")
            nc.sync.dma_start(out=gt[:], in_=dist[0:16, 32768:DCOLS])
            for k, jn in enumerate(FULL_SIZES[1:], start=1):
                ftiles.append(full_dma(k, jn, foff))
                foff += jn

            cstb_t = cpool.tile([128, CB_TOT], BF, tag="cstb")
            nc.scalar.dma_start(out=cstb_t[:], in_=cstb[:])
            cstf_t = cpool.tile([128, CF_TOT], FP, tag="cstf")
            nc.scalar.dma_start(out=cstf_t[:], in_=cstf[:])
            xT_t = cstb_t[:, CB_XT:CB_XT + ROWS]
            wcf_t = cstb_t[:, CB_WCF:CB_WCF + H]
            wdf_t = cstb_t[:, CB_WDF:CB_WDF + H]
            wfc_t = cstb_t[:, CB_WFC:CB_WFC + D]
            bcf_row = cstb_t[0:1, CB_BCFR:CB_BCFR + H]
            ones_row = cstb_t[0:1, CB_ONES:CB_ONES + ROWS]
            ident = cstf_t[:, CF_EYE:CF_EYE + ROWS]
            bdfN = cstf_t[:, CF_BDFN:CF_BDFN + 1]
            mask1 = cstf_t[0:16, CF_M1:CF_M1 + 4]
            mask2 = cstf_t[0:16, CF_M2:CF_M2 + 4]
            mask56 = cstf_t[64:120, CF_M56:CF_M56 + 28]

            # hx^T = Wcf @ x^T + bcf x ones -> (H, ROWS) in PSUM (bf16 mms)
            hx_ps = ppool.tile([H, ROWS], FP, tag="hx_ps")
            nc.tensor.matmul(hx_ps[:], wcf_t, xT_t, start=True, stop=False)
            nc.tensor.matmul(hx_ps[:], bcf_row, ones_row, start=False, stop=True)
            hxT = wpool.tile([H, ROWS], FP, tag="hxT")
            nc.scalar.copy(hxT[:], hx_ps[:])
            s0T = wpool.tile([H, ROWS], BF, tag="s0T")
            nc.scalar.mul(s0T[:], hx_ps[:], bdfN)

            # Preload the bias term (hx * N*bdf) @ Wfc^T into out^T PSUM.
            outT_ps = ppool.tile([D, ROWS], FP, tag="outT_ps")
            nc.tensor.matmul(outT_ps[:], wfc_t, s0T[:], start=True, stop=False,
                             skip_group_check=True)

            dsT_ps = ppool.tile([D, ROWS], FP, tag="dsT_ps")
            scs = [spool.tile([ROWS, 64 * max(FULL_SIZES)], BF, name=f"sc{i}",
                              tag=f"sc{i}") for i in range(2)]
            rts = [wpool.tile([ROWS, D], FP, name=f"r{i}", tag=f"r{i}")
                   for i in range(4)]

            def fold(dve, tile, ncols, prange, r):
                """halving reduction of tile[prange, :ncols] into r[prange,:].
                stage1 fp32->bf16, mid stages bf16 (2x DVE), last stage fp32."""
                p0, p1 = prange
                sc = scs[fold.flip]
                fold.flip ^= 1
                half = ncols // 2
                dve.tensor_add(sc[p0:p1, 0:half], tile[p0:p1, 0:half],
                               tile[p0:p1, half:2 * half])
                c = half // 2
                while c > D:
                    dve.tensor_add(sc[p0:p1, 0:c], sc[p0:p1, 0:c],
                                   sc[p0:p1, c:2 * c])
                    c //= 2
                dve.tensor_add(r[p0:p1, :], sc[p0:p1, 0:D], sc[p0:p1, D:2 * D])
            fold.flip = 0

            with nc.allow_low_precision("fold mid-stages in bf16; validated "
                                        "rel err 4e-3 vs 2e-2 budget"):
                # DVE in arrival order: wA, fw0, pct, gt, fw1..; PE's first
                # dsT op must be the full-width T(fw0) (start=True zeroes the
                # whole tile), later ops accumulate.
                fold(nc.vector, wa, JOFF * D, (0, 92), rts[0])
                fold(nc.vector, ftiles[0], FULL_SIZES[0] * D, (0, ROWS), rts[1])
                nc.tensor.matmul(dsT_ps[:], rts[1][:], ident,
                                 is_transpose=True, start=True, stop=False,
                                 skip_group_check=True)
                nc.tensor.matmul(dsT_ps[:, 0:92], rts[0][0:92, :],
                                 ident[0:92, 0:92], is_transpose=True,
                                 start=False, stop=False, skip_group_check=True)
                fold(nc.vector, pct, PKCOLS, (64, 120), rts[2])
                nc.tensor.matmul(dsT_ps[:, 96:124], rts[2][64:120, :], mask56,
                                 start=False, stop=False, skip_group_check=True)
                fold(nc.vector, gt, PKCOLS, (0, 16), rts[3])
                nc.tensor.matmul(dsT_ps[:, 92:96], rts[3][0:16, :], mask1,
                                 start=False, stop=False, skip_group_check=True)
                nc.tensor.matmul(dsT_ps[:, 124:128], rts[3][0:16, :], mask2,
                                 start=False, stop=False, skip_group_check=True)
                for k, jn in enumerate(FULL_SIZES[1:], start=1):
                    r = rts[k % 4]
                    fold(nc.vector, ftiles[k], jn * D, (0, ROWS), r)
                    nc.tensor.matmul(dsT_ps[:], r[:], ident,
                                     is_transpose=True, start=False,
                                     stop=(k == len(FULL_SIZES) - 1),
                                     skip_group_check=True)

            # Tail: dsT -> bf16, hd^T = Wdf @ ds^T, s^T = hx^T * hd^T,
            # out^T += Wfc @ s^T (onto preloaded bias term), store.
            # Split into i-column halves to pipeline ACT/PE/DVE stages.
            dsT_b = wpool.tile([D, ROWS], BF, tag="dsT_b")
            hd_ps = ppool.tile([H, ROWS], FP, tag="hd_ps")
            sT = wpool.tile([H, ROWS], BF, tag="sT")
            out_sb = wpool.tile([D, ROWS], FP, tag="out_sb")
            HALVES = ((0, 64), (64, 128))
            for h0, h1 in HALVES:
                nc.scalar.copy(dsT_b[:, h0:h1], dsT_ps[:, h0:h1])
            for h0, h1 in HALVES:
                nc.tensor.matmul(hd_ps[:, h0:h1], wdf_t, dsT_b[:, h0:h1],
                                 start=True, stop=True, skip_group_check=True)
            for h0, h1 in HALVES:
                nc.vector.tensor_mul(sT[:, h0:h1], hd_ps[:, h0:h1],
                                     hxT[:, h0:h1])
            for h0, h1 in HALVES:
                nc.tensor.matmul(outT_ps[:, h0:h1], wfc_t, sT[:, h0:h1],
                                 start=False, stop=(h0 == 64),
                                 skip_group_check=True)
            for hi, (h0, h1) in enumerate(HALVES):
                nc.scalar.copy(out_sb[:, h0:h1], outT_ps[:, h0:h1])
                nc.scalar.dma_start(out=out[hi * D:(hi + 1) * D, :],
                                    in_=out_sb[:, h0:h1])
    nc.compile()
    return nc


_NC_CACHE = None


def _get_nc():
    global _NC_CACHE
    if _NC_CACHE is None:
        _NC_CACHE = build_nc()
    return _NC_CACHE


def _make_in_maps(x, distance, Wcf_w, Wcf_b, Wdf_w, Wdf_b, Wfc_w):
    x = np.ascontiguousarray(np.asarray(x, np.float32))
    distance = np.ascontiguousarray(np.asarray(distance, np.float32))
    x_flat = x.reshape(B * N, D)
    dist_flat = distance.reshape(B * N, N * D)
    wcfT = np.asarray(Wcf_w, np.float32).T.astype(bfloat16)
    wdfT = np.asarray(Wdf_w, np.float32).T.astype(bfloat16)
    wfcT = np.asarray(Wfc_w, np.float32).T.astype(bfloat16)
    bcf = np.asarray(Wcf_b, np.float32).astype(bfloat16)
    bdfN = np.asarray(Wdf_b, np.float32) * float(N)
    cstf_blk = np.zeros((128, CF_TOT), np.float32)
    cstf_blk[:, CF_EYE:CF_EYE + ROWS] = np.eye(ROWS, dtype=np.float32)
    cstf_blk[:, CF_BDFN] = bdfN
    for p in range(16):
        q = p // 2
        cstf_blk[p, (CF_M1 + q) if q < 4 else (CF_M2 + q - 4)] = 1.0
    for p in range(56):
        cstf_blk[64 + p, CF_M56 + p // 2] = 1.0
    in_maps = []
    for c in range(NCORES):
        sl = slice(c * ROWS, (c + 1) * ROWS)
        dc = dist_flat[sl]  # [128, 32768]
        dblk = np.zeros((ROWS, DCOLS), np.float32)
        dblk[:, 0:N * D] = dc  # cols [0,4096)=j<32; [4096,32768)=j>=32
        for p in range(16):     # guest: slow rows' j in [0,32)
            r, s = SLOW[p // 2], p % 2
            dblk[p, 32768:DCOLS] = dc[r, s * 16 * D:(s + 1) * 16 * D]
        for p in range(56):     # packed rows 96-123, j in [0,32)
            r, s = 96 + p // 2, p % 2
            dblk[64 + p, 32768:DCOLS] = dc[r, s * 16 * D:(s + 1) * 16 * D]
        cstb_blk = np.zeros((128, CB_TOT), bfloat16)
        cstb_blk[:, CB_XT:CB_XT + ROWS] = x_flat[sl].T.astype(bfloat16)
        cstb_blk[:, CB_WCF:CB_WCF + H] = wcfT
        cstb_blk[:, CB_WDF:CB_WDF + H] = wdfT
        cstb_blk[:, CB_WFC:CB_WFC + D] = wfcT
        cstb_blk[0, CB_BCFR:CB_BCFR + H] = bcf
        cstb_blk[0, CB_ONES:CB_ONES + ROWS] = bfloat16(1.0)
        in_maps.append({
            "dist": dblk,
            "cstb": cstb_blk,
            "cstf": cstf_blk,
        })
    return in_maps


def kernel(x, distance, Wcf_w, Wcf_b, Wdf_w, Wdf_b, Wfc_w):
    in_maps = _make_in_maps(x, distance, Wcf_w, Wcf_b, Wdf_w, Wdf_b, Wfc_w)
    nc = _get_nc()
    res = run_bass_kernel_spmd(nc, in_maps, list(range(NCORES))).results
    # per-core result is out^T stored as two stacked [D, 64] i-halves
    outs = []
    for c in range(NCORES):
        a = res[c]["out"]  # [2*D, 64]
        outT = np.concatenate([a[0:D, :], a[D:2 * D, :]], axis=1)  # [D, ROWS]
        outs.append(np.ascontiguousarray(outT.T))
    return np.concatenate(outs, axis=0).reshape(B, N, D)


# revision 14
# speedup vs baseline: 1.4031x; 1.4031x over previous
"""DTNN layer kernel for Trainium2 (8 NeuronCores).

Math: out[b,i,o] = sum_j sum_h Wfc[o,h] * hx[b,i,h] * hd[b,i,j,h]
with hx = x@Wcf.T + bcf, hd = dist@Wdf.T + bdf.
Since Wfc/Wdf are linear, the j-sum commutes:
    ds[b,i,d]  = sum_j dist[b,i,j,d]                  (memory-bound reduction)
    out[b,i,:] = ((x@Wcf.T + bcf) * (ds@Wdf.T + N*bdf)) @ Wfc.T
So the kernel streams `distance` once (134MB) and does a few 128x128 matmuls.

Sharding: flatten (B,N) -> 1024 i-rows, 128 rows per core; no cross-core comms.

v2 design (from NTFF trace analysis of v1, which ran ~70-75us):
- v1's fold (DVE halving adds, ~41us busy) lagged the 43.5us DMA stream by
  ~19us because the big 64-j tiles were folded big-first (DVE idle until the
  first 4.2MB tile landed at ~17us) and the j=4 endgame tiles trickled at
  ~100GB/s (2KB per-partition lines expose HBM latency). Serial tail ~5us.
- v2: mostly-32j tiles (16KB lines, full-rate packets) tapering to 8j so the
  fold tracks arrivals; fold mid-stages run in bf16 (2x DVE rate), stage 1
  casts fp32->bf16, last stage emits a 128-col fp32 chunk result.
- Each chunk result is transposed on the (idle) PE with an accumulating
  is_transpose matmul into one PSUM tile: ds^T accumulation is free, no DVE
  acc-adds and no post-stream transpose.
- All PE matmuls run bf16 (1 cycle/row vs 4 for fp32): weights/x/biases are
  pre-cast host-side. Output is computed mirrored (out^T = WfcT^T @ sT) so
  the final matmul's stationary operand is a constant; host transposes back.
- PSUM->SBUF copies ride the Scalar(ACT) engine, keeping DVE for folds only.
Numpy-simulated rel err of this scheme: 4.1e-3 (gate 2e-2).
"""

import numpy as np
from ml_dtypes import bfloat16

import concourse.bass as bass
import concourse.bacc as bacc
import concourse.mybir as mybir
from concourse.tile import TileContext
from concourse.bass_utils import run_bass_kernel_spmd

B, N, D, H = 4, 256, 128, 128
NCORES = 8
ROWS = B * N // NCORES  # 128 i-rows per core
FP = mybir.dt.float32
BF = mybir.dt.bfloat16

# j-counts per streamed tile: 32j tiles (16KB per-partition lines = full-rate
# 16KB DMA packets) tapering to 8j so the last fold chain is short.
SIZES = [16, 32, 32, 32, 32, 32, 32, 16, 16, 8, 8]
assert sum(SIZES) == N

# bf16 constant block columns: [xT | wcfT | wdfT | wfcT | bcf_row | ones_row]
CB_XT = 0
CB_WCF = 128
CB_WDF = 256
CB_WFC = 384
CB_BCFR = 512   # partition 0: bcf row (1, H)
CB_ONES = 640   # partition 0: ones row (1, ROWS)
CB_TOT = 768

# fp32 constant block columns: [eye | N*bdf col]
CF_EYE = 0
CF_BDFN = 128   # per-partition column (H, 1) = N * bdf
CF_TOT = 129


def build_nc():
    nc = bacc.Bacc("TRN2", target_bir_lowering=False)
    dist = nc.declare_dram_parameter("dist", [ROWS, N * D], FP, isOutput=False)
    cstb = nc.declare_dram_parameter("cstb", [128, CB_TOT], BF, isOutput=False)
    cstf = nc.declare_dram_parameter("cstf", [128, CF_TOT], FP, isOutput=False)
    out = nc.declare_dram_parameter("out", [2 * D, ROWS // 2], FP,
                                    isOutput=True)

    with TileContext(nc) as tc:
        with (
            tc.tile_pool(name="const", bufs=1) as cpool,
            tc.tile_pool(name="dist", bufs=1) as dpool,
            tc.tile_pool(name="scratch", bufs=1) as spool,
            tc.tile_pool(name="work", bufs=1) as wpool,
            tc.tile_pool(name="psum", bufs=1, space="PSUM") as ppool,
        ):
            # dist stream first so the big DMAs start ASAP (sync HWDGE ring);
            # constants ride the scalar HWDGE ring concurrently.
            dtiles = []
            off = 0
            for k, jn in enumerate(SIZES):
                t = dpool.tile([ROWS, jn * D], FP, tag=f"dist{k}")
                nc.sync.dma_start(out=t[:], in_=dist[:, off * D:(off + jn) * D])
                dtiles.append(t)
                off += jn

            cstb_t = cpool.tile([128, CB_TOT], BF, tag="cstb")
            nc.scalar.dma_start(out=cstb_t[:], in_=cstb[:])
            cstf_t = cpool.tile([128, CF_TOT], FP, tag="cstf")
            nc.scalar.dma_start(out=cstf_t[:], in_=cstf[:])
            xT_t = cstb_t[:, CB_XT:CB_XT + ROWS]
            wcf_t = cstb_t[:, CB_WCF:CB_WCF + H]
            wdf_t = cstb_t[:, CB_WDF:CB_WDF + H]
            wfc_t = cstb_t[:, CB_WFC:CB_WFC + D]
            bcf_row = cstb_t[0:1, CB_BCFR:CB_BCFR + H]
            ones_row = cstb_t[0:1, CB_ONES:CB_ONES + ROWS]
            ident = cstf_t[:, CF_EYE:CF_EYE + ROWS]
            bdfN = cstf_t[:, CF_BDFN:CF_BDFN + 1]

            # hx^T = Wcf @ x^T + bcf x ones -> (H, ROWS) in PSUM (bf16 mms)
            hx_ps = ppool.tile([H, ROWS], FP, tag="hx_ps")
            nc.tensor.matmul(hx_ps[:], wcf_t, xT_t, start=True, stop=False)
            nc.tensor.matmul(hx_ps[:], bcf_row, ones_row, start=False, stop=True)
            # fp32 copy for the final DVE mul; bf16 scaled copy for the bias
            # preload term. Both on ACT, reading hx straight from PSUM.
            hxT = wpool.tile([H, ROWS], FP, tag="hxT")
            nc.scalar.copy(hxT[:], hx_ps[:])
            s0T = wpool.tile([H, ROWS], BF, tag="s0T")
            nc.scalar.mul(s0T[:], hx_ps[:], bdfN)

            # Preload the bias term (hx * N*bdf) @ Wfc^T (mirrored: into
            # out^T PSUM); the final matmul accumulates onto it.
            outT_ps = ppool.tile([D, ROWS], FP, tag="outT_ps")
            nc.tensor.matmul(outT_ps[:], wfc_t, s0T[:], start=True, stop=False,
                             skip_group_check=True)

            # Streaming j-reduction: each tile halved with DVE adds (stage 1
            # casts fp32->bf16, mid stages run bf16 at 2x rate, last stage
            # emits fp32 128 cols), then the chunk result is transposed on
            # the PE, accumulating ds^T in PSUM across chunks.
            dsT_ps = ppool.tile([D, ROWS], FP, tag="dsT_ps")
            scs = [spool.tile([ROWS, 64 * max(SIZES)], BF, name=f"sc{i}",
                              tag=f"sc{i}") for i in range(2)]
            rts = [wpool.tile([ROWS, D], FP, name=f"r{i}", tag=f"r{i}")
                   for i in range(2)]
            with nc.allow_low_precision("fold mid-stages in bf16; validated "
                                        "rel err 4e-3 vs 2e-2 budget"):
                for k, jn in enumerate(SIZES):
                    t, sc, r = dtiles[k], scs[k % 2], rts[k % 2]
                    half = jn * D // 2
                    nc.vector.tensor_add(
                        sc[:, 0:half], t[:, 0:half], t[:, half:2 * half]
                    )
                    c = half // 2
                    while c > D:
                        nc.vector.tensor_add(
                            sc[:, 0:c], sc[:, 0:c], sc[:, c:2 * c]
                        )
                        c //= 2
                    nc.vector.tensor_add(r[:], sc[:, 0:D], sc[:, D:2 * D])
                    nc.tensor.matmul(dsT_ps[:], r[:], ident,
                                     is_transpose=True, start=(k == 0),
                                     stop=(k == len(SIZES) - 1),
                                     skip_group_check=True)

            # Tail: dsT -> bf16, hd^T = Wdf @ ds^T, s^T = hx^T * hd^T,
            # out^T += Wfc @ s^T (onto preloaded bias term), store.
            dsT_b = wpool.tile([D, ROWS], BF, tag="dsT_b")
            hd_ps = ppool.tile([H, ROWS], FP, tag="hd_ps")
            sT = wpool.tile([H, ROWS], BF, tag="sT")
            out_sb = wpool.tile([D, ROWS], FP, tag="out_sb")
            HALVES = ((0, 64), (64, 128))
            for h0, h1 in HALVES:
                nc.scalar.copy(dsT_b[:, h0:h1], dsT_ps[:, h0:h1])
            for h0, h1 in HALVES:
                nc.tensor.matmul(hd_ps[:, h0:h1], wdf_t, dsT_b[:, h0:h1],
                                 start=True, stop=True, skip_group_check=True)
            for h0, h1 in HALVES:
                nc.vector.tensor_mul(sT[:, h0:h1], hd_ps[:, h0:h1],
                                     hxT[:, h0:h1])
            for h0, h1 in HALVES:
                nc.tensor.matmul(outT_ps[:, h0:h1], wfc_t, sT[:, h0:h1],
                                 start=False, stop=(h0 == 64),
                                 skip_group_check=True)
            for hi, (h0, h1) in enumerate(HALVES):
                nc.scalar.copy(out_sb[:, h0:h1], outT_ps[:, h0:h1])
                nc.scalar.dma_start(out=out[hi * D:(hi + 1) * D, :],
                                    in_=out_sb[:, h0:h1])
    nc.compile()
    return nc


_NC_CACHE = None


def _get_nc():
    global _NC_CACHE
    if _NC_CACHE is None:
        _NC_CACHE = build_nc()
    return _NC_CACHE


def _make_in_maps(x, distance, Wcf_w, Wcf_b, Wdf_w, Wdf_b, Wfc_w):
    x = np.ascontiguousarray(np.asarray(x, np.float32))
    distance = np.ascontiguousarray(np.asarray(distance, np.float32))
    x_flat = x.reshape(B * N, D)
    dist_flat = distance.reshape(B * N, N * D)
    wcfT = np.asarray(Wcf_w, np.float32).T.astype(bfloat16)
    wdfT = np.asarray(Wdf_w, np.float32).T.astype(bfloat16)
    wfcT = np.asarray(Wfc_w, np.float32).T.astype(bfloat16)
    bcf = np.asarray(Wcf_b, np.float32).astype(bfloat16)
    bdfN = (np.asarray(Wdf_b, np.float32) * float(N))
    cstf_blk = np.zeros((128, CF_TOT), np.float32)
    cstf_blk[:, CF_EYE:CF_EYE + ROWS] = np.eye(ROWS, dtype=np.float32)
    cstf_blk[:, CF_BDFN] = bdfN
    in_maps = []
    for c in range(NCORES):
        sl = slice(c * ROWS, (c + 1) * ROWS)
        cstb_blk = np.zeros((128, CB_TOT), bfloat16)
        cstb_blk[:, CB_XT:CB_XT + ROWS] = x_flat[sl].T.astype(bfloat16)
        cstb_blk[:, CB_WCF:CB_WCF + H] = wcfT
        cstb_blk[:, CB_WDF:CB_WDF + H] = wdfT
        cstb_blk[:, CB_WFC:CB_WFC + D] = wfcT
        cstb_blk[0, CB_BCFR:CB_BCFR + H] = bcf
        cstb_blk[0, CB_ONES:CB_ONES + ROWS] = bfloat16(1.0)
        in_maps.append({
            "dist": np.ascontiguousarray(dist_flat[sl]),
            "cstb": cstb_blk,
            "cstf": cstf_blk,
        })
    return in_maps


def kernel(x, distance, Wcf_w, Wcf_b, Wdf_w, Wdf_b, Wfc_w):
    in_maps = _make_in_maps(x, distance, Wcf_w, Wcf_b, Wdf_w, Wdf_b, Wfc_w)
    nc = _get_nc()
    res = run_bass_kernel_spmd(nc, in_maps, list(range(NCORES))).results
    # per-core result is out^T stored as two stacked [D, 64] i-halves
    outs = []
    for c in range(NCORES):
        a = res[c]["out"]  # [2*D, 64]
        outT = np.concatenate([a[0:D, :], a[D:2 * D, :]], axis=1)
        outs.append(np.ascontiguousarray(outT.T))
    return np.concatenate(outs, axis=0).reshape(B, N, D)


# revision 15
# speedup vs baseline: 1.4159x; 1.0091x over previous
"""DTNN layer kernel for Trainium2 (8 NeuronCores).

Math: out[b,i,o] = sum_j sum_h Wfc[o,h] * hx[b,i,h] * hd[b,i,j,h]
with hx = x@Wcf.T + bcf, hd = dist@Wdf.T + bdf.
Since Wfc/Wdf are linear, the j-sum commutes:
    ds[b,i,d]  = sum_j dist[b,i,j,d]                  (memory-bound reduction)
    out[b,i,:] = ((x@Wcf.T + bcf) * (ds@Wdf.T + N*bdf)) @ Wfc.T
So the kernel streams `distance` once (134MB) and does a few 128x128 matmuls.

Sharding: flatten (B,N) -> 1024 i-rows, 128 rows per core; no cross-core comms.

v2 design (from NTFF trace analysis of v1, which ran ~70-75us):
- v1's fold (DVE halving adds, ~41us busy) lagged the 43.5us DMA stream by
  ~19us because the big 64-j tiles were folded big-first (DVE idle until the
  first 4.2MB tile landed at ~17us) and the j=4 endgame tiles trickled at
  ~100GB/s (2KB per-partition lines expose HBM latency). Serial tail ~5us.
- v2: mostly-32j tiles (16KB lines, full-rate packets) tapering to 8j so the
  fold tracks arrivals; fold mid-stages run in bf16 (2x DVE rate), stage 1
  casts fp32->bf16, last stage emits a 128-col fp32 chunk result.
- Each chunk result is transposed on the (idle) PE with an accumulating
  is_transpose matmul into one PSUM tile: ds^T accumulation is free, no DVE
  acc-adds and no post-stream transpose.
- All PE matmuls run bf16 (1 cycle/row vs 4 for fp32): weights/x/biases are
  pre-cast host-side. Output is computed mirrored (out^T = WfcT^T @ sT) so
  the final matmul's stationary operand is a constant; host transposes back.
- PSUM->SBUF copies ride the Scalar(ACT) engine, keeping DVE for folds only.
Numpy-simulated rel err of this scheme: 4.1e-3 (gate 2e-2).
"""

import numpy as np
from ml_dtypes import bfloat16

import concourse.bass as bass
import concourse.bacc as bacc
import concourse.mybir as mybir
from concourse.tile import TileContext
from concourse.bass_utils import run_bass_kernel_spmd

B, N, D, H = 4, 256, 128, 128
NCORES = 8
ROWS = B * N // NCORES  # 128 i-rows per core
FP = mybir.dt.float32
BF = mybir.dt.bfloat16

# j-counts per streamed tile: 32j tiles (16KB per-partition lines = full-rate
# 16KB DMA packets) tapering to 8j so the last fold chain is short.
SIZES = [16, 32, 32, 32, 32, 32, 32, 16, 8, 8, 8, 8]
assert sum(SIZES) == N

# bf16 constant block columns: [xT | wcfT | wdfT | wfcT | bcf_row | ones_row]
CB_XT = 0
CB_WCF = 128
CB_WDF = 256
CB_WFC = 384
CB_BCFR = 512   # partition 0: bcf row (1, H)
CB_ONES = 640   # partition 0: ones row (1, ROWS)
CB_TOT = 768

# fp32 constant block columns: [eye | N*bdf col]
CF_EYE = 0
CF_BDFN = 128   # per-partition column (H, 1) = N * bdf
CF_TOT = 129


def build_nc():
    nc = bacc.Bacc("TRN2", target_bir_lowering=False)
    dist = nc.declare_dram_parameter("dist", [ROWS, N * D], FP, isOutput=False)
    cstb = nc.declare_dram_parameter("cstb", [128, CB_TOT], BF, isOutput=False)
    cstf = nc.declare_dram_parameter("cstf", [128, CF_TOT], FP, isOutput=False)
    out = nc.declare_dram_parameter("out", [2 * D, ROWS // 2], FP,
                                    isOutput=True)

    with TileContext(nc) as tc:
        with (
            tc.tile_pool(name="const", bufs=1) as cpool,
            tc.tile_pool(name="dist", bufs=1) as dpool,
            tc.tile_pool(name="scratch", bufs=1) as spool,
            tc.tile_pool(name="work", bufs=1) as wpool,
            tc.tile_pool(name="psum", bufs=1, space="PSUM") as ppool,
        ):
            # dist stream first so the big DMAs start ASAP (sync HWDGE ring);
            # constants ride the scalar HWDGE ring concurrently.
            dtiles = []
            off = 0
            for k, jn in enumerate(SIZES):
                t = dpool.tile([ROWS, jn * D], FP, tag=f"dist{k}")
                nc.sync.dma_start(out=t[:], in_=dist[:, off * D:(off + jn) * D])
                dtiles.append(t)
                off += jn

            cstb_t = cpool.tile([128, CB_TOT], BF, tag="cstb")
            nc.scalar.dma_start(out=cstb_t[:], in_=cstb[:])
            cstf_t = cpool.tile([128, CF_TOT], FP, tag="cstf")
            nc.scalar.dma_start(out=cstf_t[:], in_=cstf[:])
            xT_t = cstb_t[:, CB_XT:CB_XT + ROWS]
            wcf_t = cstb_t[:, CB_WCF:CB_WCF + H]
            wdf_t = cstb_t[:, CB_WDF:CB_WDF + H]
            wfc_t = cstb_t[:, CB_WFC:CB_WFC + D]
            bcf_row = cstb_t[0:1, CB_BCFR:CB_BCFR + H]
            ones_row = cstb_t[0:1, CB_ONES:CB_ONES + ROWS]
            ident = cstf_t[:, CF_EYE:CF_EYE + ROWS]
            bdfN = cstf_t[:, CF_BDFN:CF_BDFN + 1]

            # hx^T = Wcf @ x^T + bcf x ones -> (H, ROWS) in PSUM (bf16 mms)
            hx_ps = ppool.tile([H, ROWS], FP, tag="hx_ps")
            nc.tensor.matmul(hx_ps[:], wcf_t, xT_t, start=True, stop=False)
            nc.tensor.matmul(hx_ps[:], bcf_row, ones_row, start=False, stop=True)
            # fp32 copy for the final DVE mul; bf16 scaled copy for the bias
            # preload term. Both on ACT, reading hx straight from PSUM.
            hxT = wpool.tile([H, ROWS], FP, tag="hxT")
            nc.scalar.copy(hxT[:], hx_ps[:])
            s0T = wpool.tile([H, ROWS], BF, tag="s0T")
            nc.scalar.mul(s0T[:], hx_ps[:], bdfN)

            # Preload the bias term (hx * N*bdf) @ Wfc^T (mirrored: into
            # out^T PSUM); the final matmul accumulates onto it.
            outT_ps = ppool.tile([D, ROWS], FP, tag="outT_ps")
            nc.tensor.matmul(outT_ps[:], wfc_t, s0T[:], start=True, stop=False,
                             skip_group_check=True)

            # Streaming j-reduction: each tile halved with DVE adds (stage 1
            # casts fp32->bf16, mid stages run bf16 at 2x rate, last stage
            # emits fp32 128 cols), then the chunk result is transposed on
            # the PE, accumulating ds^T in PSUM across chunks.
            dsT_ps = ppool.tile([D, ROWS], FP, tag="dsT_ps")
            scs = [spool.tile([ROWS, 64 * max(SIZES)], BF, name=f"sc{i}",
                              tag=f"sc{i}") for i in range(2)]
            rts = [wpool.tile([ROWS, D], FP, name=f"r{i}", tag=f"r{i}")
                   for i in range(2)]
            with nc.allow_low_precision("fold mid-stages in bf16; validated "
                                        "rel err 4e-3 vs 2e-2 budget"):
                for k, jn in enumerate(SIZES):
                    t, sc, r = dtiles[k], scs[k % 2], rts[k % 2]
                    half = jn * D // 2
                    nc.vector.tensor_add(
                        sc[:, 0:half], t[:, 0:half], t[:, half:2 * half]
                    )
                    c = half // 2
                    while c > D:
                        nc.vector.tensor_add(
                            sc[:, 0:c], sc[:, 0:c], sc[:, c:2 * c]
                        )
                        c //= 2
                    nc.vector.tensor_add(r[:], sc[:, 0:D], sc[:, D:2 * D])
                    nc.tensor.matmul(dsT_ps[:], r[:], ident,
                                     is_transpose=True, start=(k == 0),
                                     stop=(k == len(SIZES) - 1),
                                     skip_group_check=True)

            # Tail: dsT -> bf16, hd^T = Wdf @ ds^T, s^T = hx^T * hd^T,
            # out^T += Wfc @ s^T (onto preloaded bias term), store.
            dsT_b = wpool.tile([D, ROWS], BF, tag="dsT_b")
            hd_ps = ppool.tile([H, ROWS], FP, tag="hd_ps")
            sT = wpool.tile([H, ROWS], BF, tag="sT")
            out_sb = wpool.tile([D, ROWS], FP, tag="out_sb")
            HALVES = ((0, 64), (64, 128))
            for h0, h1 in HALVES:
                nc.scalar.copy(dsT_b[:, h0:h1], dsT_ps[:, h0:h1])
            for h0, h1 in HALVES:
                nc.tensor.matmul(hd_ps[:, h0:h1], wdf_t, dsT_b[:, h0:h1],
                                 start=True, stop=True, skip_group_check=True)
            for h0, h1 in HALVES:
                nc.vector.tensor_mul(sT[:, h0:h1], hd_ps[:, h0:h1],
                                     hxT[:, h0:h1])
            for h0, h1 in HALVES:
                nc.tensor.matmul(outT_ps[:, h0:h1], wfc_t, sT[:, h0:h1],
                                 start=False, stop=(h0 == 64),
                                 skip_group_check=True)
            for hi, (h0, h1) in enumerate(HALVES):
                nc.scalar.copy(out_sb[:, h0:h1], outT_ps[:, h0:h1])
                nc.scalar.dma_start(out=out[hi * D:(hi + 1) * D, :],
                                    in_=out_sb[:, h0:h1])
    nc.compile()
    return nc


_NC_CACHE = None


def _get_nc():
    global _NC_CACHE
    if _NC_CACHE is None:
        _NC_CACHE = build_nc()
    return _NC_CACHE


def _make_in_maps(x, distance, Wcf_w, Wcf_b, Wdf_w, Wdf_b, Wfc_w):
    x = np.ascontiguousarray(np.asarray(x, np.float32))
    distance = np.ascontiguousarray(np.asarray(distance, np.float32))
    x_flat = x.reshape(B * N, D)
    dist_flat = distance.reshape(B * N, N * D)
    wcfT = np.asarray(Wcf_w, np.float32).T.astype(bfloat16)
    wdfT = np.asarray(Wdf_w, np.float32).T.astype(bfloat16)
    wfcT = np.asarray(Wfc_w, np.float32).T.astype(bfloat16)
    bcf = np.asarray(Wcf_b, np.float32).astype(bfloat16)
    bdfN = (np.asarray(Wdf_b, np.float32) * float(N))
    cstf_blk = np.zeros((128, CF_TOT), np.float32)
    cstf_blk[:, CF_EYE:CF_EYE + ROWS] = np.eye(ROWS, dtype=np.float32)
    cstf_blk[:, CF_BDFN] = bdfN
    in_maps = []
    for c in range(NCORES):
        sl = slice(c * ROWS, (c + 1) * ROWS)
        cstb_blk = np.zeros((128, CB_TOT), bfloat16)
        cstb_blk[:, CB_XT:CB_XT + ROWS] = x_flat[sl].T.astype(bfloat16)
        cstb_blk[:, CB_WCF:CB_WCF + H] = wcfT
        cstb_blk[:, CB_WDF:CB_WDF + H] = wdfT
        cstb_blk[:, CB_WFC:CB_WFC + D] = wfcT
        cstb_blk[0, CB_BCFR:CB_BCFR + H] = bcf
        cstb_blk[0, CB_ONES:CB_ONES + ROWS] = bfloat16(1.0)
        in_maps.append({
            "dist": np.ascontiguousarray(dist_flat[sl]),
            "cstb": cstb_blk,
            "cstf": cstf_blk,
        })
    return in_maps


def kernel(x, distance, Wcf_w, Wcf_b, Wdf_w, Wdf_b, Wfc_w):
    in_maps = _make_in_maps(x, distance, Wcf_w, Wcf_b, Wdf_w, Wdf_b, Wfc_w)
    nc = _get_nc()
    res = run_bass_kernel_spmd(nc, in_maps, list(range(NCORES))).results
    # per-core result is out^T stored as two stacked [D, 64] i-halves
    outs = []
    for c in range(NCORES):
        a = res[c]["out"]  # [2*D, 64]
        outT = np.concatenate([a[0:D, :], a[D:2 * D, :]], axis=1)
        outs.append(np.ascontiguousarray(outT.T))
    return np.concatenate(outs, axis=0).reshape(B, N, D)
